# revision 6
# baseline (speedup 1.0000x reference)
"""CausalVAE layer kernel for 8x Trainium2 NeuronCores (Bass/Tile).

Strategy
--------
Data-parallel over the 16384 tokens (B*S), 2048 tokens per core. All params
replicated. Feature-major activation layout on device ([feature_chunk=128
partitions, tokens in free dim]) so every matmul is transpose-free with
weights stationary on the PE:

  x^T --enc--> h^T --LN(2048)--> (mu^T -> W1(dag+concept1 fused, block-lower-
  triangular) -> per-concept LN+ReLU -> blockdiag concept2 -> dec1 ->
  LN(1024)+ReLU -> dec2 -> x_recon^T), logvar^T -> KL partial sums.

Host folds (exactly, using the actual input values):
  - DAG mix (I + tril(softplus(dag_w),-1)) into the concept-1 linear => W1eff
  - encoder LN affine (g,bt) of the mu half into W1eff columns + bias
  - all per-feature biases/affines into per-partition bias/scale columns
    consumed by ScalarE activation ops (bias/scale are [P,1] APs)
  - trace(expm(dag)) == C exactly (dag strictly lower-triangular => nilpotent,
    unit diagonal preserved exactly in fp arithmetic)

LN statistics (feature-axis = partition-axis reductions) run on the PE as
ones-vector / indicator matmuls accumulated over feature chunks; per-token
mean/rstd are broadcast back across partitions with rank-1 matmuls. Scalar
loss terms are accumulated per-partition via fused accum_out columns and
reduced on host in float64.
"""

import os
import sys

import numpy as np

sys.path.insert(0, "/opt/trn_rl_repo")

B, S, D = 8, 2048, 1024
C, CD = 16, 64
L = C * CD  # 1024
ALPHA = 0.3
LN_EPS = 1e-5

NCORES = 8
TTOT = B * S              # 16384 tokens
TCORE = TTOT // NCORES    # 2048 tokens per core
NTOK = 512                # tokens per group (matmul moving free dim)
NGROUPS = TCORE // NTOK   # 4

F32 = None  # set after mybir import

# const column layout in `cols` [128, 96]
COL_ENCB = 0      # 16 cols: encoder bias per enc out chunk
COL_C1 = 16       # 8 cols: W1eff@bt_mu + cb1 (t bias)
COL_CG = 24       # 8 cols: concept LN gain
COL_CBT = 32      # 8 cols: concept LN bias
COL_DB1 = 40      # 8 cols: dec_b1
COL_DECG = 48     # 8 cols: dec LN gain
COL_DECBT = 56    # 8 cols: dec LN bias
COL_DB2 = 64      # 8 cols: dec_b2
COL_GLV = 72      # 8 cols: enc_g logvar half
COL_BTLV = 80     # 8 cols: enc_bt logvar half
COL_CB2 = 88      # 8 cols: cb2 (zt bias)
COL_EPS = 94      # 1 col: LN epsilon
COL_GMU = 96      # 8 cols: enc_g mu half
COL_BTMU = 104    # 8 cols: enc_bt mu half
NCOLS = 112

# stats_act [128, 128]: SMU2 0..31 (g*8+k), SLV 32..63, SEXP 64..95, LOSS 96..127
SA_S2MU = 0
SA_SLV = 32
SA_SEXP = 64
SA_LOSS = 96

_compiled_nc = None


def _tri_idx(k, j):
    return k * (k + 1) // 2 + j


def build_bass():
    import concourse.bass as bass
    import concourse.mybir as mybir
    import concourse.tile as tile
    from concourse import bacc

    f32 = mybir.dt.float32
    AF = mybir.ActivationFunctionType
    OP = mybir.AluOpType

    nc = bacc.Bacc()

    # ---- per-core DRAM params ----
    xT = nc.declare_dram_parameter("xT", [D, TCORE], f32, isOutput=False)
    encwT = nc.declare_dram_parameter("encwT", [D, 2 * L], f32, isOutput=False)
    w1g = nc.declare_dram_parameter("w1g", [36, 128, 128], f32, isOutput=False)
    w2blk = nc.declare_dram_parameter("w2blk", [8, 128, 128], f32, isOutput=False)
    d1T = nc.declare_dram_parameter("d1T", [L, L], f32, isOutput=False)
    d2T = nc.declare_dram_parameter("d2T", [L, D], f32, isOutput=False)
    cols_d = nc.declare_dram_parameter("cols", [128, NCOLS], f32, isOutput=False)
    onesc_d = nc.declare_dram_parameter("onesc", [128, 1], f32, isOutput=False)
    onesr_d = nc.declare_dram_parameter("onesr", [1, 128], f32, isOutput=False)
    ind_d = nc.declare_dram_parameter("ind", [128, 8, 16], f32, isOutput=False)
    indT_d = nc.declare_dram_parameter("indT", [16, 8, 128], f32, isOutput=False)
    negI_d = nc.declare_dram_parameter("negI", [128, 128], f32, isOutput=False)

    xr_out = nc.declare_dram_parameter("xr_out", [D, TCORE], f32, isOutput=True)
    stats_act_out = nc.declare_dram_parameter(
        "stats_act_out", [128, 128], f32, isOutput=True
    )

    with tile.TileContext(nc) as tc:
        with (
            tc.tile_pool(name="const", bufs=1) as cpool,
            tc.tile_pool(name="xp", bufs=1) as xpool,
            tc.tile_pool(name="hp", bufs=1) as hpool,
            tc.tile_pool(name="lat", bufs=2) as lat,
            tc.tile_pool(name="wenc", bufs=4) as wencp,
            tc.tile_pool(name="scr", bufs=3) as scr,
            tc.tile_pool(name="small", bufs=3) as small,
            tc.tile_pool(name="pmm", bufs=3, space="PSUM") as pmm,
            tc.tile_pool(name="pstat", bufs=2, space="PSUM") as pstat,
            tc.tile_pool(name="pbc", bufs=3, space="PSUM") as pbc,
        ):
            # ---------- constants ----------
            w1g_sb = cpool.tile([128, 36, 128], f32, tag="w1g")
            for i in range(36):
                nc.sync.dma_start(out=w1g_sb[:, i, :], in_=w1g[i])
            w2_sb = cpool.tile([128, 8, 128], f32, tag="w2")
            for i in range(8):
                nc.sync.dma_start(out=w2_sb[:, i, :], in_=w2blk[i])
            d1_sb = cpool.tile([128, 8, L], f32, tag="d1")
            for j in range(8):
                nc.sync.dma_start(
                    out=d1_sb[:, j, :], in_=d1T[j * 128 : (j + 1) * 128, :]
                )
            d2_sb = cpool.tile([128, 8, D], f32, tag="d2")
            for j in range(8):
                nc.sync.dma_start(
                    out=d2_sb[:, j, :], in_=d2T[j * 128 : (j + 1) * 128, :]
                )
            cols = cpool.tile([128, NCOLS], f32, tag="cols")
            nc.sync.dma_start(out=cols[:], in_=cols_d[:])
            onesc = cpool.tile([128, 1], f32, tag="onesc")
            nc.sync.dma_start(out=onesc[:], in_=onesc_d[:])
            onesr = cpool.tile([1, 128], f32, tag="onesr")
            nc.sync.dma_start(out=onesr[:], in_=onesr_d[:])
            ind_sb = cpool.tile([128, 8, 16], f32, tag="ind")
            nc.sync.dma_start(out=ind_sb[:], in_=ind_d[:])
            indT_sb = cpool.tile([16, 8, 128], f32, tag="indT")
            nc.sync.dma_start(out=indT_sb[:], in_=indT_d[:])
            negI_sb = cpool.tile([128, 128], f32, tag="negI")
            nc.sync.dma_start(out=negI_sb[:], in_=negI_d[:])

            stats_act = cpool.tile([128, 128], f32, tag="sact")

            def col(i):
                return cols[:, i : i + 1]

            for g in range(NGROUPS):
                tsl = slice(g * NTOK, (g + 1) * NTOK)

                x_sb = xpool.tile([128, 8, NTOK], f32, tag="x")
                nc.sync.dma_start(
                    out=x_sb[:], in_=xT.rearrange("(j p) t -> p j t", p=128)[:, :, tsl]
                )

                # ---------- encoder: h^T = encW^T-chunks.T @ x^T ----------
                h_sb = hpool.tile([128, 16, NTOK], f32, tag="h")
                pS1 = pstat.tile([1, NTOK], f32, tag="pstat")
                pS2 = pstat.tile([1, NTOK], f32, tag="pstat")
                for kc in range(16):
                    pm = pmm.tile([128, NTOK], f32, tag="pmm")
                    w = wencp.tile([128, 8, 128], f32, tag="wenc")
                    nc.sync.dma_start(
                        out=w[:],
                        in_=encwT.rearrange("(j p) k -> p j k", p=128)[
                            :, :, kc * 128 : (kc + 1) * 128
                        ],
                    )
                    for j in range(8):
                        nc.tensor.matmul(
                            pm[:], w[:, j, :], x_sb[:, j, :], start=(j == 0), stop=(j == 7)
                        )
                    # evacuate with encoder bias; squared copy for variance
                    nc.vector.tensor_scalar_add(
                        h_sb[:, kc, :], pm[:], col(COL_ENCB + kc)
                    )
                    h2 = scr.tile([128, NTOK], f32, tag="scr")
                    nc.scalar.activation(
                        h2[:], pm[:], AF.Square, bias=col(COL_ENCB + kc), scale=1.0
                    )
                    nc.tensor.matmul(
                        pS1[:], onesc[:], h_sb[:, kc, :],
                        start=(kc == 0), stop=(kc == 15),
                    )
                    nc.tensor.matmul(
                        pS2[:], onesc[:], h2[:], start=(kc == 0), stop=(kc == 15)
                    )

                # ---------- encoder LN stats ----------
                mneg = small.tile([1, NTOK], f32, tag="sm")
                nc.scalar.activation(
                    mneg[:], pS1[:], AF.Copy, bias=0.0, scale=-1.0 / 2048.0
                )
                m2 = small.tile([1, NTOK], f32, tag="sm")
                nc.scalar.activation(
                    m2[:], pS1[:], AF.Square, bias=0.0, scale=1.0 / 2048.0
                )
                var = small.tile([1, NTOK], f32, tag="sm")
                nc.vector.scalar_tensor_tensor(
                    var[:], pS2[:], 1.0 / 2048.0, m2[:], OP.mult, OP.subtract
                )
                sd = small.tile([1, NTOK], f32, tag="sm")
                nc.scalar.activation(sd[:], var[:], AF.Sqrt, bias=cols[0:1, COL_EPS:COL_EPS+1], scale=1.0)
                shat = small.tile([1, NTOK], f32, tag="sm")
                nc.vector.reciprocal(shat[:], sd[:])

                pmb = pbc.tile([128, NTOK], f32, tag="pbc")
                nc.tensor.matmul(pmb[:], onesr[:], mneg[:])
                psb = pbc.tile([128, NTOK], f32, tag="pbc")
                nc.tensor.matmul(psb[:], onesr[:], shat[:])

                # ---------- mu half -> n (normalized, affine folded into W1) ----
                n_sb = lat.tile([128, 8, NTOK], f32, tag="lat")
                for kc in range(8):
                    tmp = scr.tile([128, NTOK], f32, tag="scr")
                    nc.vector.tensor_add(tmp[:], h_sb[:, kc, :], pmb[:])
                    nc.vector.tensor_mul(n_sb[:, kc, :], tmp[:], psb[:])
                    sq = scr.tile([128, NTOK], f32, tag="scr")
                    # (g_mu*n + bt_mu)^2 summed over tokens -> Sum(mu^2) partials
                    nc.scalar.activation(
                        sq[:], n_sb[:, kc, :], AF.Square,
                        bias=col(COL_BTMU + kc), scale=col(COL_GMU + kc),
                        accum_out=stats_act[:, SA_S2MU + g * 8 + kc : SA_S2MU + g * 8 + kc + 1],
                    )

                # ---------- logvar half -> KL partials ----------
                for i in range(8):
                    kc = 8 + i
                    tmp = scr.tile([128, NTOK], f32, tag="scr")
                    nc.vector.tensor_add(tmp[:], h_sb[:, kc, :], pmb[:])
                    tmp2 = scr.tile([128, NTOK], f32, tag="scr")
                    nc.vector.tensor_mul(tmp2[:], tmp[:], psb[:])
                    lv = scr.tile([128, NTOK], f32, tag="scr")
                    # logvar = g_lv*tmp2 + bt_lv; accumulate Sum(logvar)
                    nc.scalar.activation(
                        lv[:], tmp2[:], AF.Identity,
                        bias=col(COL_BTLV + i), scale=col(COL_GLV + i),
                        accum_out=stats_act[:, SA_SLV + g * 8 + i : SA_SLV + g * 8 + i + 1],
                    )
                    ex = scr.tile([128, NTOK], f32, tag="scr")
                    nc.scalar.activation(
                        ex[:], lv[:], AF.Exp, bias=0.0, scale=1.0,
                        accum_out=stats_act[:, SA_SEXP + g * 8 + i : SA_SEXP + g * 8 + i + 1],
                    )

                # ---------- W1 (dag + concept1 + enc affine), block-lower-tri ----
                t_sb = lat.tile([128, 8, NTOK], f32, tag="lat")
                pcS1 = pstat.tile([16, NTOK], f32, tag="pstat")
                pcS2 = pstat.tile([16, NTOK], f32, tag="pstat")
                for k in range(8):
                    pm = pmm.tile([128, NTOK], f32, tag="pmm")
                    for j in range(k + 1):
                        nc.tensor.matmul(
                            pm[:], w1g_sb[:, _tri_idx(k, j), :], n_sb[:, j, :],
                            start=(j == 0), stop=(j == k),
                        )
                    nc.vector.tensor_scalar_add(t_sb[:, k, :], pm[:], col(COL_C1 + k))
                    t2 = scr.tile([128, NTOK], f32, tag="scr")
                    nc.scalar.activation(
                        t2[:], pm[:], AF.Square, bias=col(COL_C1 + k), scale=1.0
                    )
                    nc.tensor.matmul(
                        pcS1[:], ind_sb[:, k, :], t_sb[:, k, :],
                        start=(k == 0), stop=(k == 7),
                    )
                    nc.tensor.matmul(
                        pcS2[:], ind_sb[:, k, :], t2[:], start=(k == 0), stop=(k == 7)
                    )

                # ---------- concept LN stats ([16, NTOK]) ----------
                mnegc = small.tile([16, NTOK], f32, tag="smc")
                nc.scalar.activation(
                    mnegc[:], pcS1[:], AF.Copy, bias=0.0, scale=-1.0 / 64.0
                )
                m2c = small.tile([16, NTOK], f32, tag="smc")
                nc.scalar.activation(
                    m2c[:], pcS1[:], AF.Square, bias=0.0, scale=1.0 / 64.0
                )
                varc = small.tile([16, NTOK], f32, tag="smc")
                nc.vector.scalar_tensor_tensor(
                    varc[:], pcS2[:], 1.0 / 64.0, m2c[:], OP.mult, OP.subtract
                )
                sdc = small.tile([16, NTOK], f32, tag="smc")
                nc.scalar.activation(sdc[:], varc[:], AF.Sqrt, bias=cols[0:16, COL_EPS:COL_EPS+1], scale=1.0)
                shc = small.tile([16, NTOK], f32, tag="smc")
                nc.vector.reciprocal(shc[:], sdc[:])

                tpp = lat.tile([128, 8, NTOK], f32, tag="lat")
                for k in range(8):
                    pmbc = pbc.tile([128, NTOK], f32, tag="pbc")
                    nc.tensor.matmul(pmbc[:], indT_sb[:, k, :], mnegc[:])
                    psbc = pbc.tile([128, NTOK], f32, tag="pbc")
                    nc.tensor.matmul(psbc[:], indT_sb[:, k, :], shc[:])
                    tmp = scr.tile([128, NTOK], f32, tag="scr")
                    nc.vector.tensor_add(tmp[:], t_sb[:, k, :], pmbc[:])
                    tmp2 = scr.tile([128, NTOK], f32, tag="scr")
                    nc.vector.tensor_mul(tmp2[:], tmp[:], psbc[:])
                    nc.scalar.activation(
                        tpp[:, k, :], tmp2[:], AF.Relu,
                        bias=col(COL_CBT + k), scale=col(COL_CG + k),
                    )

                # ---------- concept2 (block-diagonal) ----------
                zt_sb = lat.tile([128, 8, NTOK], f32, tag="lat")
                for k in range(8):
                    pm = pmm.tile([128, NTOK], f32, tag="pmm")
                    nc.tensor.matmul(pm[:], w2_sb[:, k, :], tpp[:, k, :])
                    nc.vector.tensor_scalar_add(zt_sb[:, k, :], pm[:], col(COL_CB2 + k))

                # ---------- dec1 + LN(1024) ----------
                d_sb = lat.tile([128, 8, NTOK], f32, tag="lat")
                pdS1 = pstat.tile([1, NTOK], f32, tag="pstat")
                pdS2 = pstat.tile([1, NTOK], f32, tag="pstat")
                for k in range(8):
                    pm = pmm.tile([128, NTOK], f32, tag="pmm")
                    for j in range(8):
                        nc.tensor.matmul(
                            pm[:], d1_sb[:, j, k * 128 : (k + 1) * 128],
                            zt_sb[:, j, :], start=(j == 0), stop=(j == 7),
                        )
                    nc.vector.tensor_scalar_add(d_sb[:, k, :], pm[:], col(COL_DB1 + k))
                    d2s = scr.tile([128, NTOK], f32, tag="scr")
                    nc.scalar.activation(
                        d2s[:], pm[:], AF.Square, bias=col(COL_DB1 + k), scale=1.0
                    )
                    nc.tensor.matmul(
                        pdS1[:], onesc[:], d_sb[:, k, :], start=(k == 0), stop=(k == 7)
                    )
                    nc.tensor.matmul(
                        pdS2[:], onesc[:], d2s[:], start=(k == 0), stop=(k == 7)
                    )

                mnegd = small.tile([1, NTOK], f32, tag="sm")
                nc.scalar.activation(
                    mnegd[:], pdS1[:], AF.Copy, bias=0.0, scale=-1.0 / 1024.0
                )
                m2d = small.tile([1, NTOK], f32, tag="sm")
                nc.scalar.activation(
                    m2d[:], pdS1[:], AF.Square, bias=0.0, scale=1.0 / 1024.0
                )
                vard = small.tile([1, NTOK], f32, tag="sm")
                nc.vector.scalar_tensor_tensor(
                    vard[:], pdS2[:], 1.0 / 1024.0, m2d[:], OP.mult, OP.subtract
                )
                sdd = small.tile([1, NTOK], f32, tag="sm")
                nc.scalar.activation(sdd[:], vard[:], AF.Sqrt, bias=cols[0:1, COL_EPS:COL_EPS+1], scale=1.0)
                shd = small.tile([1, NTOK], f32, tag="sm")
                nc.vector.reciprocal(shd[:], sdd[:])

                pmbd = pbc.tile([128, NTOK], f32, tag="pbc")
                nc.tensor.matmul(pmbd[:], onesr[:], mnegd[:])
                psbd = pbc.tile([128, NTOK], f32, tag="pbc")
                nc.tensor.matmul(psbd[:], onesr[:], shd[:])

                dpp = lat.tile([128, 8, NTOK], f32, tag="lat")
                for k in range(8):
                    tmp = scr.tile([128, NTOK], f32, tag="scr")
                    nc.vector.tensor_add(tmp[:], d_sb[:, k, :], pmbd[:])
                    tmp2 = scr.tile([128, NTOK], f32, tag="scr")
                    nc.vector.tensor_mul(tmp2[:], tmp[:], psbd[:])
                    nc.scalar.activation(
                        dpp[:, k, :], tmp2[:], AF.Relu,
                        bias=col(COL_DECBT + k), scale=col(COL_DECG + k),
                    )

                # ---------- dec2 + output + recon-loss partials ----------
                xr_sb = lat.tile([128, 8, NTOK], f32, tag="lat")
                for k in range(8):
                    pm = pmm.tile([128, NTOK], f32, tag="pmm")
                    for j in range(8):
                        nc.tensor.matmul(
                            pm[:], d2_sb[:, j, k * 128 : (k + 1) * 128],
                            dpp[:, j, :], start=(j == 0), stop=(j == 7),
                        )
                    nc.vector.tensor_scalar_add(xr_sb[:, k, :], pm[:], col(COL_DB2 + k))
                    nc.sync.dma_start(
                        out=xr_out[k * 128 : (k + 1) * 128, tsl], in_=xr_sb[:, k, :]
                    )
                    # pm += -x  (after the evacuation read; Tile orders via WAR)
                    nc.tensor.matmul(
                        pm[:], negI_sb[:], x_sb[:, k, :], start=False, stop=True,
                        skip_group_check=True,
                    )
                    lsq = scr.tile([128, NTOK], f32, tag="scr")
                    nc.scalar.activation(
                        lsq[:], pm[:], AF.Square, bias=col(COL_DB2 + k), scale=1.0,
                        accum_out=stats_act[:, SA_LOSS + g * 8 + k : SA_LOSS + g * 8 + k + 1],
                    )

            nc.sync.dma_start(out=stats_act_out[:], in_=stats_act[:])

    nc.compile()
    return nc


def _get_nc():
    global _compiled_nc
    if _compiled_nc is None:
        _compiled_nc = build_bass()
    return _compiled_nc


def _prep_host(inputs):
    f = np.float32
    x = np.asarray(inputs["x"], f)
    enc_w = np.asarray(inputs["enc_w"], f)
    enc_b = np.asarray(inputs["enc_b"], f)
    enc_g = np.asarray(inputs["enc_g"], f)
    enc_bt = np.asarray(inputs["enc_bt"], f)
    dag_w = np.asarray(inputs["dag_w"], f)
    cw1 = np.asarray(inputs["cw1"], f)
    cb1 = np.asarray(inputs["cb1"], f)
    cg = np.asarray(inputs["cg"], f)
    cbt = np.asarray(inputs["cbt"], f)
    cw2 = np.asarray(inputs["cw2"], f)
    cb2 = np.asarray(inputs["cb2"], f)
    dec_w1 = np.asarray(inputs["dec_w1"], f)
    dec_b1 = np.asarray(inputs["dec_b1"], f)
    dec_g = np.asarray(inputs["dec_g"], f)
    dec_bt = np.asarray(inputs["dec_bt"], f)
    dec_w2 = np.asarray(inputs["dec_w2"], f)
    dec_b2 = np.asarray(inputs["dec_b2"], f)

    xT = np.ascontiguousarray(x.reshape(TTOT, D).T)  # [D, TTOT]

    sp = np.logaddexp(np.float32(0.0), dag_w).astype(f)  # softplus, f32
    dag = np.tril(sp, k=-1)
    M1 = np.eye(C, dtype=f) + dag

    # W1eff[c*64+d, j*64+e] = M1[c,j] * cw1[c,d,e]
    W1eff = np.einsum("cj,cde->cdje", M1, cw1).reshape(L, L).astype(f)
    g_mu, g_lv = enc_g[:L], enc_g[L:]
    bt_mu, bt_lv = enc_bt[:L], enc_bt[L:]
    W1g = (W1eff * g_mu[None, :]).astype(f)
    c1 = (W1eff @ bt_mu + cb1.reshape(-1)).astype(f)

    w1g_packed = np.zeros((36, 128, 128), f)
    for k in range(8):
        for j in range(k + 1):
            w1g_packed[_tri_idx(k, j)] = W1g[
                k * 128 : (k + 1) * 128, j * 128 : (j + 1) * 128
            ].T

    w2blk = np.zeros((8, 128, 128), f)
    for k in range(8):
        w2blk[k, :64, :64] = cw2[2 * k].T
        w2blk[k, 64:, 64:] = cw2[2 * k + 1].T

    d1T = np.ascontiguousarray(dec_w1.T)
    d2T = np.ascontiguousarray(dec_w2.T)

    cols = np.zeros((128, NCOLS), f)
    for kc in range(16):
        cols[:, COL_ENCB + kc] = enc_b[kc * 128 : (kc + 1) * 128]
    for k in range(8):
        sl = slice(k * 128, (k + 1) * 128)
        cols[:, COL_C1 + k] = c1[sl]
        cols[:, COL_CG + k] = cg.reshape(-1)[sl]
        cols[:, COL_CBT + k] = cbt.reshape(-1)[sl]
        cols[:, COL_DB1 + k] = dec_b1[sl]
        cols[:, COL_DECG + k] = dec_g[sl]
        cols[:, COL_DECBT + k] = dec_bt[sl]
        cols[:, COL_DB2 + k] = dec_b2[sl]
        cols[:, COL_GLV + k] = g_lv[sl]
        cols[:, COL_BTLV + k] = bt_lv[sl]
        cols[:, COL_CB2 + k] = cb2.reshape(-1)[sl]
        cols[:, COL_GMU + k] = g_mu[sl]
        cols[:, COL_BTMU + k] = bt_mu[sl]
    cols[:, COL_EPS] = LN_EPS

    onesc = np.ones((128, 1), f)
    onesr = np.ones((1, 128), f)
    ind = np.zeros((128, 8, 16), f)
    indT = np.zeros((16, 8, 128), f)
    for k in range(8):
        for p in range(128):
            c = 2 * k + (1 if p >= 64 else 0)
            ind[p, k, c] = 1.0
            indT[c, k, p] = 1.0
    negI = (-np.eye(128)).astype(f)

    shared = dict(
        encwT=np.ascontiguousarray(enc_w.T),
        w1g=w1g_packed, w2blk=w2blk, d1T=d1T, d2T=d2T, cols=cols,
        onesc=onesc, onesr=onesr, ind=ind, indT=indT, negI=negI,
    )
    in_maps = []
    for cidx in range(NCORES):
        m = dict(shared)
        m["xT"] = np.ascontiguousarray(xT[:, cidx * TCORE : (cidx + 1) * TCORE])
        in_maps.append(m)

    aux = dict(
        x=x, dag=dag, sp=sp, g_mu=g_mu, g_lv=g_lv, bt_mu=bt_mu, bt_lv=bt_lv,
    )
    return in_maps, aux


def _combine(results, aux):
    f64 = np.float64
    # x_recon: per-core [D, TCORE] feature-major -> [B,S,D]
    xrT = np.concatenate([np.asarray(r["xr_out"]) for r in results], axis=1)
    x_recon = np.ascontiguousarray(xrT.T).reshape(B, S, D).astype(np.float32)

    sact = np.stack([np.asarray(r["stats_act_out"]) for r in results]).astype(f64)

    sum_mu2 = sact[:, :, SA_S2MU : SA_S2MU + 32].sum()
    sum_lv = sact[:, :, SA_SLV : SA_SLV + 32].sum()
    Sexp = sact[:, :, SA_SEXP : SA_SEXP + 32].sum()
    Sloss = sact[:, :, SA_LOSS : SA_LOSS + 32].sum()

    T = float(TTOT)
    nelem = T * L
    kl = -0.5 * (1.0 + (sum_lv - sum_mu2 - Sexp) / nelem)
    recon = Sloss / (T * D)
    dag_loss = float(np.tril(aux["sp"], k=-1).sum(dtype=f64)) + float(C)
    total = recon + ALPHA * kl + dag_loss

    return (
        x_recon,
        np.float32(total),
        np.float32(kl),
        np.float32(recon),
        np.float32(dag_loss),
    )


def kernel(**inputs):
    from concourse.bass_utils import run_bass_kernel_spmd

    nc = _get_nc()
    in_maps, aux = _prep_host(inputs)
    res = run_bass_kernel_spmd(nc, in_maps, list(range(NCORES)))
    return _combine(res.results, aux)


if __name__ == "__main__":
    # smoke test with random data
    rng = np.random.default_rng(0)
    ins = {
        "x": rng.standard_normal((B, S, D), np.float32),
        "enc_w": rng.standard_normal((2 * L, D), np.float32) * 0.02,
        "enc_b": np.zeros(2 * L, np.float32),
        "enc_g": np.ones(2 * L, np.float32),
        "enc_bt": np.zeros(2 * L, np.float32),
        "dag_w": rng.standard_normal((C, C), np.float32) * 0.1,
        "cw1": rng.standard_normal((C, CD, CD), np.float32) * 0.02,
        "cb1": np.zeros((C, CD), np.float32),
        "cg": np.ones((C, CD), np.float32),
        "cbt": np.zeros((C, CD), np.float32),
        "cw2": rng.standard_normal((C, CD, CD), np.float32) * 0.02,
        "cb2": np.zeros((C, CD), np.float32),
        "dec_w1": rng.standard_normal((L, L), np.float32) * 0.02,
        "dec_b1": np.zeros(L, np.float32),
        "dec_g": np.ones(L, np.float32),
        "dec_bt": np.zeros(L, np.float32),
        "dec_w2": rng.standard_normal((D, L), np.float32) * 0.02,
        "dec_b2": np.zeros(D, np.float32),
    }
    out = kernel(**ins)
    print([np.asarray(o).shape for o in out])
